# revision 3
# baseline (speedup 1.0000x reference)
"""Multi-head attention Bass/Tile kernel v2 for Trainium2 (8 cores, SPMD).

Reference semantics (note the reference's intentional name swap):
    k = split_heads(query @ Wk.T); q = split_heads(key @ Wq.T)
    v = split_heads(value @ Wv.T)
    wei = softmax(where(mask==0, 0, (q @ k^T) * C**-0.5))
    out = (wei @ v) -> merge heads -> @ Wproj.T + bproj

v2 design vs v1:
  * S^T computed per head-PAIR with two concurrent row-tiled matmuls
    (head even on PE rows 0-63, head odd on rows 64-127) into one
    [128, 2, 512] PSUM pair tile -> ~2x S throughput.
  * exp done as ONE wide ACT instruction per (kb, q-chunk) spanning both
    heads' PSUM banks -> amortizes the per-instruction ACT overhead.
  * PV is flipped: stationary = V' block [128, 65] (64 feats + ones col),
    moving = e-tile -> output u^T [65, qw] PSUM with the softmax
    denominator in row 64.  No per-matmul reload of a 128-col stationary.
  * masking-with-zero handled algebraically as in v1:
      p = m*(exp(s)-1) + 1; fully-masked blocks contribute colsum(V')
    which is added per-q-block as a per-partition scalar in the
    normalize step (DVE scalar_tensor_tensor), not as PE rank-1 matmuls.
  * normalization: denominators DMA'd into a [96, 128] layout
    (partition = head*16+qt), reciprocal'd in one DVE op per pair, and
    DMA-broadcast back to [64, t] per head; then one fused DVE op per
    (head, qt): out = (u^T + C) * (1/den)  written straight into the
    output-projection stationary layout.  Zero PE transposes.
  * output projection reads the normalized u^T directly (stationary),
    y = sum_p u^T_p.T @ WprojT_p + b.
"""

import os
import sys

sys.path.insert(0, "/opt/trn_rl_repo")

import numpy as np

B, T, C = 8, 2048, 384
H, D = 6, 64
VW = D + 1  # per-head V width incl. ones column (softmax denominator)
SCALE = float(C) ** -0.5
P = 128
QC = 512  # q-chunk width (one PSUM bank of fp32 per plane)

ZERO, ONES, MIXED = 0, 1, 2

_CACHE = {}
LAST_PROFILE = {}


def _runs(bools):
    """Contiguous True runs in a list -> [(start, end)) pairs."""
    runs, s = [], None
    for i, b in enumerate(bools):
        if b and s is None:
            s = i
        elif not b and s is not None:
            runs.append((s, i))
            s = None
    if s is not None:
        runs.append((s, len(bools)))
    return runs


def _build_program(t, cls, mixed_ids, n_mixed, repeat=1, debug=False):
    from contextlib import ExitStack

    import concourse.mybir as mybir
    import concourse.tile as tile
    from concourse import bacc

    f32 = mybir.dt.float32
    f32r = mybir.dt.float32r
    bf16 = mybir.dt.bfloat16
    Alu = mybir.AluOpType
    Act = mybir.ActivationFunctionType

    nt = t // P
    nqc = t // QC
    BPC = QC // P  # q-blocks per chunk
    PAIRS = H // 2

    # per-qt correction sets (blocks whose masked part contributes colsum(V'))
    corr_kbs = [[kb for kb in range(nt) if cls[kb][qt] != ONES] for qt in range(nt)]
    # per-chunk live structure: chunk_live[qc] = [(kb, [(r0, r1), ...])]
    chunk_live = []
    for qc in range(nqc):
        lst = []
        for kb in range(nt):
            liv = [cls[kb][qc * BPC + d] != ZERO for d in range(BPC)]
            rs = [(r0 * P, r1 * P) for (r0, r1) in _runs(liv)]
            if rs:
                lst.append((kb, rs))
        # widest total coverage first so the first matmul's start=True
        # (which zeroes the whole PSUM bank) covers later partial writes
        lst.sort(key=lambda e: -(sum(r1 - r0 for r0, r1 in e[1])))
        chunk_live.append(lst)

    nc = bacc.Bacc()

    xqT = nc.dram_tensor("xqT", [C, t], f32r, kind="ExternalInput")
    xkT = nc.dram_tensor("xkT", [C, t], f32r, kind="ExternalInput")
    xvT = nc.dram_tensor("xvT", [C, t], f32r, kind="ExternalInput")
    wkT = nc.dram_tensor("wkT", [C, C], f32r, kind="ExternalInput")
    wqT = nc.dram_tensor("wqT", [C, C], f32r, kind="ExternalInput")
    wvT = nc.dram_tensor("wvT", [C, C], f32r, kind="ExternalInput")
    wpT = nc.dram_tensor("wpT", [C, C], f32, kind="ExternalInput")
    bpj = nc.dram_tensor("bpj", [1, C], f32, kind="ExternalInput")
    if n_mixed:
        mmT = nc.dram_tensor("mmT", [n_mixed, P, P], bf16, kind="ExternalInput")
    y = nc.dram_tensor("y", [t, C], f32, kind="ExternalOutput")
    dbg = {}
    if debug:
        for name, shape, dt in (
            ("dbg_qT", [P, 3, t], bf16),
            ("dbg_kT", [P, 3, t], bf16),
            ("dbg_vp", [P, nt, H * VW], bf16),
            ("dbg_cT", [VW, H, nt], f32),
            ("dbg_nmask", [H * nt, 1], f32),
            ("dbg_denP", [H * nt, P], bf16),
            ("dbg_rcP", [H * nt, P], f32),
            ("dbg_pair", [P, 3, t], bf16),
            ("dbg_e000", [P, 2, QC], bf16),
            ("dbg_us00", [VW, QC], bf16),
            ("dbg_e100", [P, 2, QC], bf16),
            ("dbg_us20", [VW, QC], bf16),
        ):
            dbg[name] = nc.dram_tensor(name, shape, dt, kind="ExternalOutput")

    with ExitStack() as ctx:
        tc = ctx.enter_context(tile.TileContext(nc))
        if repeat > 1:
            ctx.enter_context(tc.For_i(0, repeat, 1))
        consts = ctx.enter_context(tc.tile_pool(name="consts", bufs=1))
        dscr = ctx.enter_context(tc.tile_pool(name="dscr", bufs=1, space="DRAM"))
        cs_dram = dscr.tile([nt, H * VW], f32, tag="cs")
        den_dram = dscr.tile([H * nt, P], bf16, tag="den")
        rc_dram = dscr.tile([H * nt, P], bf16, tag="rc")

        # ---- persistent tiles ------------------------------------------
        ones_col = consts.tile([P, 1], bf16, tag="ones_col")
        nc.vector.memset(ones_col, 1.0)
        bias_s = consts.tile([P, C], f32, tag="bias")
        nc.sync.dma_start(out=bias_s, in_=bpj[:, :].to_broadcast((P, C)))
        if n_mixed:
            mm_s = consts.tile([P, n_mixed, P], bf16, tag="mm")
            nc.sync.dma_start(out=mm_s, in_=mmT[:, :, :].rearrange("n p f -> p n f"))
        qT_s = consts.tile([P, PAIRS, t], bf16, tag="qT")
        kT_s = consts.tile([P, PAIRS, t], bf16, tag="kT")
        vp_s = consts.tile([P, nt, H * VW], bf16, tag="vp")
        wp_bf = consts.tile([P, 3, C], bf16, tag="wp_bf")
        c_T = consts.tile([VW, H, nt], f32, tag="c_T")
        nmask_s = consts.tile([H * nt, 1], f32, tag="nmask")
        denP = consts.tile([H * nt, P], bf16, tag="denP")
        denF = consts.tile([H * nt, P], f32, tag="denF")
        rcP = consts.tile([H * nt, P], f32, tag="rcP")
        rcPb = consts.tile([H * nt, P], bf16, tag="rcPb")
        nc.vector.memset(denP, 1.0)
        pair_us = consts.tile([P, PAIRS, t], bf16, tag="pair_us")

        # ---- phase B pools (allocated first: pools release in LIFO
        # order and these outlive the phase A pools) ---------------------
        phB = ExitStack()
        spool = phB.enter_context(tc.tile_pool(name="spsum", bufs=2, space="PSUM"))
        epool = phB.enter_context(tc.tile_pool(name="epool", bufs=20))
        uspool = phB.enter_context(tc.tile_pool(name="uspool", bufs=10))
        stpool = phB.enter_context(tc.tile_pool(name="stage", bufs=2))
        rcb_pool = phB.enter_context(tc.tile_pool(name="rcb", bufs=2))
        sc_pool = phB.enter_context(tc.tile_pool(name="scr", bufs=4))
        ut_holder = {}  # psum pool created after phase A psum pools close

        # ---- phase A: projections --------------------------------------
        paA = ExitStack()
        xt_pool = paA.enter_context(tc.tile_pool(name="xt", bufs=1))
        w_pool = paA.enter_context(tc.tile_pool(name="wqkv", bufs=1))
        pproj = paA.enter_context(tc.tile_pool(name="pproj", bufs=2, space="PSUM"))
        vproj = paA.enter_context(tc.tile_pool(name="vproj", bufs=1, space="PSUM"))
        cps_pool = paA.enter_context(tc.tile_pool(name="cps", bufs=1, space="PSUM"))

        w_s = {}
        for name, dram in (("wq", wqT), ("wk", wkT)):
            tl = w_pool.tile([P, 3, C], f32r, tag=name, name=f"w_{name}")
            for c in range(3):
                nc.sync.dma_start(out=tl[:, c, :], in_=dram[c * P : (c + 1) * P, :])
            w_s[name] = tl
        xs = {}
        for name, dram in (("xk", xkT), ("xq", xqT)):
            tl = xt_pool.tile([P, 3, t], f32r, tag=name, name=name)
            for c in range(3):
                nc.sync.dma_start(out=tl[:, c, :], in_=dram[c * P : (c + 1) * P, :])
            xs[name] = tl

        def proj_chunk(wname, xname, dst, i):
            # dst[:, i, :] = (W.T @ x)[i*P:(i+1)*P, :]  in QC-col chunks
            for j in range(nqc):
                ps = pproj.tile([P, QC], f32, tag="pp")
                for c in range(3):
                    nc.tensor.matmul(
                        ps,
                        lhsT=w_s[wname][:, c, i * P : (i + 1) * P],
                        rhs=xs[xname][:, c, j * QC : (j + 1) * QC],
                        start=(c == 0),
                        stop=(c == 2),
                    )
                nc.vector.tensor_copy(out=dst[:, i, j * QC : (j + 1) * QC], in_=ps)

        # reference swap: Q comes from the `key` input, K from `query`
        proj_chunk("wq", "xk", qT_s, 0)
        proj_chunk("wk", "xq", kT_s, 0)

        e_tiles = {}  # (p, qc, kb) -> tile

        def emit_S_exp(p, qc, kb, rs):
            sp = spool.tile([P, 2, QC], f32, tag="s", name=f"s_{p}_{qc}_{kb}")
            et = epool.tile([P, 2, QC], bf16, tag="e", name=f"e_{p}_{qc}_{kb}")
            for r0, r1 in rs:
                for j in range(2):
                    hs = j * D
                    nc.tensor.matmul(
                        sp[:, j, r0:r1],
                        lhsT=kT_s[hs : hs + D, p, kb * P : (kb + 1) * P],
                        rhs=qT_s[hs : hs + D, p, qc * QC + r0 : qc * QC + r1],
                        start=True,
                        stop=True,
                    )
            for r0, r1 in rs:
                nc.scalar.activation(
                    out=et[:, :, r0:r1],
                    in_=sp[:, :, r0:r1],
                    func=Act.Exp,
                    scale=SCALE,
                )
            # mixed blocks: p~ = (exp(s)-1)*m in place
            for d in range(BPC):
                qt = qc * BPC + d
                if cls[kb][qt] == MIXED:
                    for j in range(2):
                        sc = sc_pool.tile([P, 1], f32, tag="sc")
                        nc.vector.affine_mul_reduce(
                            out=et[:, j, d * P : (d + 1) * P],
                            accum_out=sc,
                            in0=et[:, j, d * P : (d + 1) * P],
                            in1=mm_s[:, mixed_ids[(kb, qt)], :],
                            scale=1.0,
                            bias=-1.0,
                        )
            e_tiles[(p, qc, kb)] = et
            if debug and (p, qc, kb) == (0, 0, 0):
                nc.sync.dma_start(out=dbg["dbg_e000"][:, :, :], in_=et)
            if debug and (p, qc, kb) == (1, 0, 0):
                nc.sync.dma_start(out=dbg["dbg_e100"][:, :, :], in_=et)

        def emit_pv(p, qc, j):
            h = 2 * p + j
            ut = ut_holder["pool"].tile([VW, QC], f32, tag="ut", name=f"ut_{h}_{qc}")
            lst = chunk_live[qc]
            if not lst:
                us = uspool.tile([VW, QC], bf16, tag="us", name=f"us_{h}_{qc}")
                nc.vector.memset(us, 0.0)
                nc.sync.dma_start(
                    out=den_dram[h * nt + qc * BPC : h * nt + qc * BPC + BPC, :],
                    in_=us[D : D + 1, :],
                )
                return us
            n = len(lst)
            for i, (kb, rs) in enumerate(lst):
                et = e_tiles[(p, qc, kb)]
                for k, (r0, r1) in enumerate(rs):
                    nc.tensor.matmul(
                        ut[:, r0:r1],
                        lhsT=vp_s[:, kb, h * VW : (h + 1) * VW],
                        rhs=et[:, j, r0:r1],
                        start=(i == 0 and k == 0),
                        stop=(i == n - 1 and k == len(rs) - 1),
                        skip_group_check=True,
                    )
            us = uspool.tile([VW, QC], bf16, tag="us", name=f"us_{h}_{qc}")
            nc.vector.tensor_copy(out=us, in_=ut)
            if debug and (h, qc) == (0, 0):
                nc.sync.dma_start(out=dbg["dbg_us00"][:, :], in_=us)
            if debug and (h, qc) == (2, 0):
                nc.sync.dma_start(out=dbg["dbg_us20"][:, :], in_=us)
            # denominator row -> den_dram rows h*nt+qt (row-major reshape)
            nc.sync.dma_start(
                out=den_dram[h * nt + qc * BPC : h * nt + qc * BPC + BPC, :],
                in_=us[D : D + 1, :],
            )
            return us

        def emit_pair_tail(p, us_tiles):
            # us_tiles: {(j, qc): us}
            r0, r1 = 2 * p * nt, (2 * p + 2) * nt
            nc.sync.dma_start(out=denP[r0:r1, :], in_=den_dram[r0:r1, :])
            # ops run on the full partition range (base partition 0):
            # partition-offset DVE ops were observed to misbehave on HW
            nc.vector.tensor_scalar_add(
                out=denF, in0=denP, scalar1=nmask_s
            )
            nc.vector.reciprocal_approx_fast(out=rcP, in_=denF)
            nc.vector.tensor_copy(out=rcPb, in_=rcP)
            nc.sync.dma_start(out=rc_dram[r0:r1, :], in_=rcPb[r0:r1, :])
            rcb = {}
            for j in range(2):
                h = 2 * p + j
                rcb[j] = rcb_pool.tile([D, t], bf16, tag="rcb", name=f"rcb_{h}")
                for qt in range(nt):
                    nc.sync.dma_start(
                        out=rcb[j][:, qt * P : (qt + 1) * P],
                        in_=rc_dram[h * nt + qt : h * nt + qt + 1, :].to_broadcast(
                            (D, P)
                        ),
                    )
            for qc in range(nqc):
                for j in range(2):
                    h = 2 * p + j
                    us = us_tiles[(j, qc)]
                    if j == 0:
                        outs = [
                            pair_us[0:D, p, (qc * BPC + d) * P : (qc * BPC + d + 1) * P]
                            for d in range(BPC)
                        ]
                    else:
                        st = stpool.tile([D, QC], bf16, tag="st", name=f"st_{h}_{qc}")
                        outs = [st[:, d * P : (d + 1) * P] for d in range(BPC)]
                    for d in range(BPC):
                        qt = qc * BPC + d
                        nc.vector.scalar_tensor_tensor(
                            out=outs[d],
                            in0=us[0:D, d * P : (d + 1) * P],
                            scalar=c_T[0:D, h, qt : qt + 1],
                            in1=rcb[j][:, qt * P : (qt + 1) * P],
                            op0=Alu.add,
                            op1=Alu.mult,
                        )
                    if j == 1:
                        nc.sync.dma_start(
                            out=pair_us[D:P, p, qc * QC : (qc + 1) * QC], in_=st
                        )

        # ---- emission loop with deferred (pending) work ----------------
        from collections import deque

        pending = deque()

        def drain(k):
            for _ in range(k):
                if pending:
                    pending.popleft()()

        phaseA_left = ["v", "qk1", "qk2", "corr", "closeA"]

        def emit_phaseA_step():
            step = phaseA_left.pop(0)
            if step == "v":
                tl = w_pool.tile([P, 3, C], f32r, tag="wv", name="w_wv")
                for c in range(3):
                    nc.sync.dma_start(
                        out=tl[:, c, :], in_=wvT[c * P : (c + 1) * P, :]
                    )
                w_s["wv"] = tl
                xv = xt_pool.tile([P, 3, t], f32r, tag="xv", name="xv")
                for c in range(3):
                    nc.sync.dma_start(
                        out=xv[:, c, :], in_=xvT[c * P : (c + 1) * P, :]
                    )
                for tt in range(nt):
                    ps = vproj.tile([P, C], f32, tag="ppv")
                    for c in range(3):
                        nc.tensor.matmul(
                            ps,
                            lhsT=xv[:, c, tt * P : (tt + 1) * P],
                            rhs=w_s["wv"][:, c, :],
                            start=(c == 0),
                            stop=(c == 2),
                        )
                    nc.vector.tensor_copy(
                        out=vp_s[:, tt, :].rearrange("p (h w) -> p h w", h=H)[
                            :, :, 0:D
                        ],
                        in_=ps.rearrange("p (h d) -> p h d", h=H),
                    )
                nc.vector.memset(
                    vp_s.rearrange("p n (h w) -> p n h w", h=H)[:, :, :, D : D + 1],
                    1.0,
                )
                # wproj: load f32 and convert to bf16
                wpf = w_pool.tile([P, 3, C], f32, tag="wpf", name="w_wpf")
                for c in range(3):
                    nc.sync.dma_start(
                        out=wpf[:, c, :], in_=wpT[c * P : (c + 1) * P, :]
                    )
                nc.vector.tensor_copy(out=wp_bf, in_=wpf)
            elif step == "qk1":
                proj_chunk("wq", "xk", qT_s, 1)
                proj_chunk("wk", "xq", kT_s, 1)
            elif step == "qk2":
                proj_chunk("wq", "xk", qT_s, 2)
                proj_chunk("wk", "xq", kT_s, 2)
            elif step == "corr":
                order = sorted(
                    (qt for qt in range(nt) if corr_kbs[qt]),
                    key=lambda q: len(corr_kbs[q]),
                )
                if not order:
                    nc.vector.memset(c_T, 0.0)
                    nc.vector.memset(nmask_s, 0.0)
                    return
                with tc.tile_pool(name="cstage", bufs=1) as cst_pool:
                    zt = cst_pool.tile([1, H * VW], f32, tag="cz")
                    nc.vector.memset(zt, 0.0)
                    for qt in range(nt):
                        if qt not in set(order):
                            nc.sync.dma_start(out=cs_dram[qt : qt + 1, :], in_=zt)
                    prev = None
                    c_ps = None
                    for qt in order:
                        s = set(corr_kbs[qt])
                        if prev is not None and prev <= s:
                            add = sorted(s - prev)
                            fresh = False
                        else:
                            add = sorted(s)
                            fresh = True
                            c_ps = cps_pool.tile(
                                [1, H * VW], f32, tag="cps", name=f"cps{qt}"
                            )
                        for i, kb in enumerate(add):
                            nc.tensor.matmul(
                                c_ps,
                                lhsT=ones_col,
                                rhs=vp_s[:, kb, :],
                                start=(fresh and i == 0),
                                stop=(i == len(add) - 1),
                                skip_group_check=True,
                            )
                        cst = cst_pool.tile(
                            [1, H * VW], f32, tag="cst", name=f"cst{qt}"
                        )
                        nc.vector.tensor_copy(out=cst, in_=c_ps)
                        nc.sync.dma_start(out=cs_dram[qt : qt + 1, :], in_=cst)
                        prev = s
                # transpose the tiny correction table into per-partition
                # layouts via small strided DMAs
                for h in range(H):
                    nc.sync.dma_start_transpose(
                        out=c_T[:, h, :], in_=cs_dram[:, h * VW : (h + 1) * VW]
                    )
                nc.sync.dma_start_transpose(
                    out=nmask_s,
                    in_=cs_dram.rearrange("q (h w) -> q h w", h=H)[:, :, D : D + 1],
                )
            elif step == "closeA":
                paA.close()
                ut_holder["pool"] = phB.enter_context(
                    tc.tile_pool(name="utpsum", bufs=4, space="PSUM")
                )

        for p in range(PAIRS):
            us_tiles = {}
            for qc in range(nqc):
                lst = chunk_live[qc]
                for kb, rs in lst:
                    emit_S_exp(p, qc, kb, rs)
                    if phaseA_left:
                        emit_phaseA_step()
                    else:
                        drain(1)
                while phaseA_left:
                    emit_phaseA_step()

                def mk(p=p, qc=qc, us_tiles=us_tiles):
                    def f0():
                        us_tiles[(0, qc)] = emit_pv(p, qc, 0)

                    def f1():
                        us_tiles[(1, qc)] = emit_pv(p, qc, 1)

                    return f0, f1

                pending.extend(mk())
            pending.append(
                lambda p=p, us_tiles=us_tiles: emit_pair_tail(p, us_tiles)
            )
        drain(len(pending))
        if debug:
            nc.sync.dma_start(out=dbg["dbg_qT"][:, :, :], in_=qT_s)
            nc.sync.dma_start(out=dbg["dbg_kT"][:, :, :], in_=kT_s)
            nc.sync.dma_start(out=dbg["dbg_vp"][:, :, :], in_=vp_s)
            nc.sync.dma_start(out=dbg["dbg_cT"][:, :, :], in_=c_T)
            nc.sync.dma_start(out=dbg["dbg_nmask"][:, :], in_=nmask_s)
            nc.sync.dma_start(out=dbg["dbg_denP"][:, :], in_=denP)
            nc.sync.dma_start(out=dbg["dbg_rcP"][:, :], in_=rcP)
            nc.sync.dma_start(out=dbg["dbg_pair"][:, :, :], in_=pair_us)
        phB.close()

        # ---- phase C: output projection --------------------------------
        with tc.tile_pool(name="ypsum", bufs=2, space="PSUM") as ypool, tc.tile_pool(
            name="ysb", bufs=3
        ) as ysb_pool:
            for qt in range(nt):
                yp = ypool.tile([P, C], f32, tag="y")
                for cc in range(PAIRS):
                    nc.tensor.matmul(
                        yp,
                        lhsT=pair_us[:, cc, qt * P : (qt + 1) * P],
                        rhs=wp_bf[:, cc, :],
                        start=(cc == 0),
                        stop=(cc == PAIRS - 1),
                    )
                ysb = ysb_pool.tile([P, C], f32, tag="ysb")
                nc.vector.tensor_add(out=ysb, in0=yp, in1=bias_s)
                nc.sync.dma_start(out=y[qt * P : (qt + 1) * P, :], in_=ysb)

    nc.finalize()
    return nc


def _classify_mask(mask2d, t):
    """Host-side classification of the [t, t] 0/1 mask into 128x128 blocks.

    Returns cls[kb][qt], per-(kb,qt) unique-block index map, and the packed
    transposed bf16 mixed blocks ([n_unique, 128, 128], m^T layout [k, q])."""
    import ml_dtypes

    nt = t // P
    m = mask2d.reshape(nt, P, nt, P)  # [qt, qp, kb, kp]
    any_ = m.any(axis=(1, 3))
    all_ = m.all(axis=(1, 3))
    cls = [[ZERO] * nt for _ in range(nt)]
    mixed_ids = {}
    blocks = []
    block_keys = {}
    for kb in range(nt):
        for qt in range(nt):
            if all_[qt, kb]:
                cls[kb][qt] = ONES
            elif any_[qt, kb]:
                cls[kb][qt] = MIXED
                blk = np.ascontiguousarray(m[qt, :, kb, :].T)
                key = blk.tobytes()
                if key not in block_keys:
                    block_keys[key] = len(blocks)
                    blocks.append(blk.astype(ml_dtypes.bfloat16))
                mixed_ids[(kb, qt)] = block_keys[key]
    packed = np.stack(blocks) if blocks else None
    return cls, mixed_ids, packed


def _make_in_maps(inputs, packed):
    query = np.asarray(inputs["query"], dtype=np.float32)
    key = np.asarray(inputs["key"], dtype=np.float32)
    value = np.asarray(inputs["value"], dtype=np.float32)
    b, t, c = query.shape
    wk = np.ascontiguousarray(np.asarray(inputs["Wk"], np.float32).T)
    wq = np.ascontiguousarray(np.asarray(inputs["Wq"], np.float32).T)
    wv = np.ascontiguousarray(np.asarray(inputs["Wv"], np.float32).T)
    wp = np.ascontiguousarray(np.asarray(inputs["Wproj"], np.float32).T)
    bp = np.asarray(inputs["bproj"], np.float32).reshape(1, c)

    in_maps = []
    for i in range(b):
        m = {
            "xqT": np.ascontiguousarray(query[i].T),
            "xkT": np.ascontiguousarray(key[i].T),
            "xvT": np.ascontiguousarray(value[i].T),
            "wkT": wk,
            "wqT": wq,
            "wvT": wv,
            "wpT": wp,
            "bpj": bp,
        }
        if packed is not None:
            m["mmT"] = packed
        in_maps.append(m)
    return in_maps


def kernel(query, key, value, mask, Wk, Wq, Wv, Wproj, bproj):
    from concourse.bass_utils import run_bass_kernel_spmd

    query = np.asarray(query, dtype=np.float32)
    b, t, c = query.shape
    mask2d = np.asarray(mask, dtype=np.int32).reshape(t, t) != 0

    cls, mixed_ids, packed = _classify_mask(mask2d, t)
    n_mixed = 0 if packed is None else len(packed)

    cache_key = (t, bytes(bytearray(v for row in cls for v in row)))
    if cache_key not in _CACHE:
        _CACHE[cache_key] = _build_program(t, cls, mixed_ids, n_mixed)
    nc = _CACHE[cache_key]

    in_maps = _make_in_maps(
        {"query": query, "key": key, "value": value, "Wk": Wk, "Wq": Wq,
         "Wv": Wv, "Wproj": Wproj, "bproj": bproj},
        packed if n_mixed else None,
    )

    trace = bool(int(os.environ.get("BASS_MHA_TRACE", "0")))
    res = run_bass_kernel_spmd(nc, in_maps, core_ids=list(range(b)), trace=trace)
    LAST_PROFILE.clear()
    LAST_PROFILE.update(
        exec_time_ns=res.exec_time_ns,
        mean_exec_time_ns=res.mean_exec_time_ns,
        trace=res.instructions_and_trace,
    )
    return np.stack([res.results[i]["y"] for i in range(b)])
